# revision 5
# baseline (speedup 1.0000x reference)
"""HGT conv layer on 8 trn2 NeuronCores.

Strategy: destination-shard the node space across the 8 cores (each core owns
1/8 of the user nodes and 1/8 of the computer nodes). Each relation's edges
are routed to the core owning their destination node and sorted by local
destination. On device, edges are processed in 128-edge groups: h_src rows
are fetched with indirect DMA, K/V computed with PE matmuls, per-edge Q
selected from the (contiguous) per-tile Q block via a one-hot permutation
matmul, and the segment softmax accumulation (exp-weighted V + exp sums) is
done as one-hot matmuls accumulating in PSUM per 128-node destination tile.
No inter-core collectives are needed; outputs are gathered on the host.
"""

import os
import sys
import types
import numpy as np

# --- axon profile hook shim (harmless when unused) ---
try:
    import antenv
    if "antenv.axon_hooks" not in sys.modules:
        _hooks = types.ModuleType("antenv.axon_hooks")
        _hooks._hook = None
        _hooks.set_axon_ntff_profile_hook = lambda h: setattr(_hooks, "_hook", h)
        _hooks.get_axon_ntff_profile_hook = lambda: _hooks._hook
        sys.modules["antenv.axon_hooks"] = _hooks
        antenv.axon_hooks = _hooks
except Exception:
    pass

from concourse import bass, bacc, mybir, tile
from concourse.bass_utils import run_bass_kernel_spmd

F32 = mybir.dt.float32
I32 = mybir.dt.int32
AX = mybir.AxisListType
OP = mybir.AluOpType
AF = mybir.ActivationFunctionType

P = 128
HID = 256
NH = 8
HD = 32
SCALE = HD ** -0.5
EPS_LN = 1e-5
NCORES = int(os.environ.get("KERNEL_NCORES", "8"))


def _sigmoid(x):
    return 1.0 / (1.0 + np.exp(-np.float64(x)))


# ----------------------------------------------------------------------------
# Host-side: shard, sort, schedule
# ----------------------------------------------------------------------------

def _prep_relation(src, dst, ew, dshard, ncores):
    """Route edges to cores by dst shard, sort by local dst, group into
    128-edge groups, assign groups to dst tiles, build the union schedule.

    Returns (K, slots, per_core) where K[t] = slot count for tile t,
    slots = [(t, j)] in emission order, and per_core[c] = dict with arrays
    src_i32 [S,128], dstl_f32 [S,128], ew_f32 [S,128] aligned to slots.
    """
    tcnt = (dshard + P - 1) // P
    core_lists = []
    for c in range(ncores):
        m = (dst >= c * dshard) & (dst < (c + 1) * dshard)
        s_, d_, w_ = src[m].astype(np.int64), (dst[m] - c * dshard).astype(np.int64), ew[m]
        o = np.argsort(d_, kind="stable")
        s_, d_, w_ = s_[o], d_[o], w_[o]
        e = len(d_)
        g = (e + P - 1) // P
        pad = g * P - e
        s_ = np.concatenate([s_, np.zeros(pad, np.int64)])
        w_ = np.concatenate([w_, np.zeros(pad, np.float64)])
        dp = np.concatenate([d_, np.full(pad, -1, np.int64)])
        tiles = [[] for _ in range(tcnt)]
        for gi in range(g):
            dd = dp[gi * P:(gi + 1) * P]
            real = dd[dd >= 0]
            assert len(real) > 0
            t_hi = int(real.max()) // P
            t_lo = int(real.min()) // P
            assert t_hi - t_lo <= 1, f"group spans {t_lo}..{t_hi}"
            if tiles[t_hi]:
                # non-first group of a tile must lie entirely inside it
                assert t_lo == t_hi
            tiles[t_hi].append((s_[gi * P:(gi + 1) * P], dd, w_[gi * P:(gi + 1) * P]))
        core_lists.append(tiles)

    K = [max(1, max(len(core_lists[c][t]) for c in range(ncores))) for t in range(tcnt)]
    slots = [(t, j) for t in range(tcnt) for j in range(K[t])]
    ns = len(slots)

    per_core = []
    pad_s = np.zeros(P, np.int64)
    pad_d = np.full(P, -1, np.int64)
    pad_w = np.zeros(P, np.float64)
    for c in range(ncores):
        src_a = np.zeros((ns, P), np.int32)
        dst_a = np.zeros((ns, P), np.float32)
        ew_a = np.zeros((ns, P), np.float32)
        for si, (t, j) in enumerate(slots):
            if j < len(core_lists[c][t]):
                s_, d_, w_ = core_lists[c][t][j]
            else:
                s_, d_, w_ = pad_s, pad_d, pad_w
            src_a[si] = s_.astype(np.int32)
            dst_a[si] = d_.astype(np.float32)
            ew_a[si] = w_.astype(np.float32)
        per_core.append({"src": src_a, "dstl": dst_a, "ew": ew_a})
    return K, slots, per_core


def _pack_blocks(per_core):
    """Pack per-slot arrays into 16-slot blocks for efficient DMA.

    srcb [B,128,16] i32, metab [B,128,16,2] f32 (dstl, ew)."""
    out = []
    for d in per_core:
        ns = d["src"].shape[0]
        b = (ns + 15) // 16
        srcb = np.zeros((b, P, 16), np.int32)
        metab = np.zeros((b, P, 16, 2), np.float32)
        for s in range(ns):
            srcb[s // 16, :, s % 16] = d["src"][s]
            metab[s // 16, :, s % 16, 0] = d["dstl"][s]
            metab[s // 16, :, s % 16, 1] = d["ew"][s]
        out.append({"srcb": srcb, "metab": metab})
    return out


# ----------------------------------------------------------------------------
# Device-side emitter
# ----------------------------------------------------------------------------

class Pools:
    def __init__(self, nc, tc, ctx):
        import contextlib
        self.nc = nc
        ep = ctx.enter_context
        self.const = ep(tc.tile_pool(name="const", bufs=1))
        self.blk = ep(tc.tile_pool(name="blk", bufs=2))
        self.hsrc = ep(tc.tile_pool(name="hsrc", bufs=4))
        self.work = ep(tc.tile_pool(name="work", bufs=2))
        self.pl = ep(tc.tile_pool(name="pl", bufs=3))
        self.qt = ep(tc.tile_pool(name="qt", bufs=3))
        self.small = ep(tc.tile_pool(name="small", bufs=3))
        self.ps_agg = ep(tc.tile_pool(name="ps_agg", bufs=2, space="PSUM"))
        self.ps_kv = ep(tc.tile_pool(name="ps_kv", bufs=2, space="PSUM"))
        self.ps_tr = ep(tc.tile_pool(name="ps_tr", bufs=2, space="PSUM"))
        self.ps_qe = ep(tc.tile_pool(name="ps_qe", bufs=1, space="PSUM"))
        self.ps_q = ep(tc.tile_pool(name="ps_q", bufs=1, space="PSUM"))

    def tr_tile(self):
        tr_ps = self.ps_tr.tile([P, 2, P], F32, tag="tr")
        return tr_ps


def _emit_transpose_256(nc, po, identity, src_ap, dst_sbuf_ap):
    """Transpose a [128, 256] sbuf block into dst [128, 2, 128] sbuf."""
    ps = po.tr_tile()
    for b in range(2):
        nc.tensor.transpose(out=ps[:, b, :], in_=src_ap[:, b * P:(b + 1) * P], identity=identity[:])
    nc.scalar.activation(out=dst_sbuf_ap[:], in_=ps[:], func=AF.Copy)


def _emit_weight_prep(nc, po, identity, wins):
    """Load + transpose weights. Returns dict of resident sbuf tiles."""
    res = {}
    for name, dram in wins["raw"].items():
        t = po.const.tile([P, 2, HID], F32, tag=f"raw_{name}")
        for b in range(2):
            nc.sync.dma_start(out=t[:, b, :], in_=dram[b * P:(b + 1) * P, :])
        res[f"raw_{name}"] = t
    # transposed weights
    for name in wins["transpose"]:
        src = res[f"raw_{name}"]
        dst = po.const.tile([P, 2, HID], F32, tag=f"T_{name}")
        for a in range(2):        # dst-dim block of original
            for b in range(2):    # input-dim block (partitions of dst)
                ps = po.tr_tile()
                nc.tensor.transpose(out=ps[:, 0, :], in_=src[:, a, b * P:(b + 1) * P], identity=identity[:])
                nc.scalar.activation(out=dst[:, b, a * P:(a + 1) * P], in_=ps[:, 0, :], func=AF.Copy)
        res[f"T_{name}"] = dst
    # fused [kwT | vwT] per relation
    for r in (1, 2, 3):
        kv = po.const.tile([P, 2, 2 * HID], F32, tag=f"kvwT{r}")
        for b in range(2):
            nc.vector.tensor_copy(out=kv[:, b, 0:HID], in_=res[f"T_kw{r}"][:, b, :])
            nc.vector.tensor_copy(out=kv[:, b, HID:2 * HID], in_=res[f"T_vw{r}"][:, b, :])
        res[f"kvwT{r}"] = kv
    # MT_r = s_r * (pw_r @ mw_r)^T, MT[i, o] = s * sum_m mw[m, i] pw[o, m]
    for r, pw_name, s_r in wins["mt"]:
        mw = res[f"raw_mw{r}"]
        pwt = res[f"T_{pw_name}"]
        mt = po.const.tile([P, 2, HID], F32, tag=f"MT{r}")
        for a in range(2):  # i-block (partitions of MT)
            ps = po.ps_q.tile([P, HID], F32, tag="q")
            for b in range(2):  # m-block (contraction)
                nc.tensor.matmul(
                    out=ps[:], lhsT=mw[:, b, a * P:(a + 1) * P], rhs=pwt[:, b, :],
                    start=(b == 0), stop=(b == 1))
            nc.scalar.activation(out=mt[:, a, :], in_=ps[:], func=AF.Copy, scale=float(s_r))
        res[f"MT{r}"] = mt
    return res


def _emit_pass(nc, po, identity, iota_rep, rel):
    """Emit one relation pass."""
    K = rel["K"]
    tcnt = len(K)
    h_src = rel["h_src"]          # dram AP source node table [N, 256]
    h_dst = rel["h_dst"]          # dram AP dst node table (full)
    dst_base = rel["dst_base"]    # row offset of this core's shard
    dst_rows = rel["dst_rows"]    # rows in shard
    qwT = rel["qwT"]
    kvwT = rel["kvwT"]
    mt = rel["MT"]
    srcb = rel["srcb"]            # dram [B, 128, 16] i32
    metab = rel["metab"]          # dram [B, 128, 16, 2] f32
    delta = rel["delta"]          # dram [tcnt*128, 256] f32 scratch

    qt_tiles = {}
    iota_tiles = {}
    agg_tiles = {}
    sb_t = None
    mb_t = None
    slot_idx = 0

    def finalize(t):
        ag = agg_tiles.pop(t)
        esum = po.small.tile([P, NH], F32, tag="esum")
        nc.vector.tensor_scalar(out=esum[:], in0=ag[:, HID:HID + NH], scalar1=1e-30,
                                scalar2=None, op0=OP.add)
        rec = po.small.tile([P, NH], F32, tag="rec")
        nc.vector.reciprocal(out=rec[:], in_=esum[:])
        wv = po.work.tile([P, HID], F32, tag="wv")
        nc.vector.tensor_tensor(
            out=wv[:].rearrange("p (h d) -> p h d", h=NH),
            in0=ag[:, 0:HID].rearrange("p (h d) -> p h d", h=NH),
            in1=rec[:].rearrange("p (h o) -> p h o", h=NH).to_broadcast([P, NH, HD]),
            op=OP.mult)
        wvT = po.work.tile([P, 2, P], F32, tag="wvT")
        _emit_transpose_256(nc, po, identity, wv[:], wvT[:])
        dps = po.ps_q.tile([P, HID], F32, tag="q")
        for b in range(2):
            nc.tensor.matmul(out=dps[:], lhsT=wvT[:, b, :], rhs=mt[:, b, :],
                             start=(b == 0), stop=(b == 1))
        dsb = po.work.tile([P, HID], F32, tag="dsb")
        nc.scalar.activation(out=dsb[:], in_=dps[:], func=AF.Copy)
        nc.sync.dma_start(out=delta[t * P:(t + 1) * P, :], in_=dsb[:])

    for t in range(tcnt):
        it = po.small.tile([P, P], F32, tag="iota")
        nc.vector.tensor_scalar(out=it[:], in0=iota_rep[:], scalar1=float(t * P),
                                scalar2=None, op0=OP.add)
        iota_tiles[t] = it
        # Q_t from contiguous dst rows
        dh = po.work.tile([P, HID], F32, tag="dh")
        rows = min(P, dst_rows - t * P)
        if rows < P:
            nc.vector.memset(dh[:], 0.0)
        nc.sync.dma_start(out=dh[:rows, :], in_=h_dst[dst_base + t * P: dst_base + t * P + rows, :])
        dhT = po.work.tile([P, 2, P], F32, tag="dhT")
        _emit_transpose_256(nc, po, identity, dh[:], dhT[:])
        qps = po.ps_q.tile([P, HID], F32, tag="q")
        for b in range(2):
            nc.tensor.matmul(out=qps[:], lhsT=dhT[:, b, :], rhs=qwT[:, b, :],
                             start=(b == 0), stop=(b == 1))
        qt = po.qt.tile([P, HID], F32, tag="qt")
        nc.scalar.activation(out=qt[:], in_=qps[:], func=AF.Copy)
        qt_tiles[t] = qt

        agg = po.ps_agg.tile([P, HID + NH], F32, tag="agg")
        agg_tiles[t] = agg

        for j in range(K[t]):
            blk, col = slot_idx // 16, slot_idx % 16
            if col == 0:
                sb_t = po.blk.tile([P, 16], I32, tag="srcb")
                nc.sync.dma_start(out=sb_t[:], in_=srcb[blk])
                mb_t = po.blk.tile([P, 16, 2], F32, tag="metab")
                nc.sync.dma_start(out=mb_t[:], in_=metab[blk])
            slot_idx += 1

            hs = po.hsrc.tile([P, HID], F32, tag="hsrc")
            nc.gpsimd.indirect_dma_start(
                out=hs[:], out_offset=None, in_=h_src[:],
                in_offset=bass.IndirectOffsetOnAxis(ap=sb_t[:, col:col + 1], axis=0))
            hT = po.work.tile([P, 2, P], F32, tag="hT")
            _emit_transpose_256(nc, po, identity, hs[:], hT[:])
            kv = po.ps_kv.tile([P, 2 * HID], F32, tag="kv")
            for b in range(2):
                nc.tensor.matmul(out=kv[:], lhsT=hT[:, b, :], rhs=kvwT[:, b, :],
                                 start=(b == 0), stop=(b == 1))

            boundary = (j == 0 and t > 0)
            pcur = po.work.tile([P, P], F32, tag="pcur")
            nc.vector.tensor_tensor(
                out=pcur[:], in0=mb_t[:, col, 0:1].to_broadcast([P, P]),
                in1=iota_tiles[t][:], op=OP.is_equal)
            pt_ps = po.tr_tile()
            nc.tensor.transpose(out=pt_ps[:, 0, :], in_=pcur[:], identity=identity[:])
            ptcur = po.work.tile([P, P], F32, tag="ptcur")
            nc.scalar.activation(out=ptcur[:], in_=pt_ps[:, 0, :], func=AF.Copy)
            if boundary:
                pprev = po.work.tile([P, P], F32, tag="pprev")
                nc.vector.tensor_tensor(
                    out=pprev[:], in0=mb_t[:, col, 0:1].to_broadcast([P, P]),
                    in1=iota_tiles[t - 1][:], op=OP.is_equal)
                pt2_ps = po.tr_tile()
                nc.tensor.transpose(out=pt2_ps[:, 0, :], in_=pprev[:], identity=identity[:])
                ptprev = po.work.tile([P, P], F32, tag="ptprev")
                nc.scalar.activation(out=ptprev[:], in_=pt2_ps[:, 0, :], func=AF.Copy)

            qe = po.ps_qe.tile([P, HID], F32, tag="qe")
            nc.tensor.matmul(out=qe[:], lhsT=ptcur[:], rhs=qt_tiles[t][:],
                             start=True, stop=not boundary)
            if boundary:
                nc.tensor.matmul(out=qe[:], lhsT=ptprev[:], rhs=qt_tiles[t - 1][:],
                                 start=False, stop=True)

            ksb = po.work.tile([P, HID], F32, tag="ksb")
            nc.scalar.activation(out=ksb[:], in_=kv[:, 0:HID], func=AF.Copy)
            qk = po.work.tile([P, HID], F32, tag="qk")
            nc.vector.tensor_tensor(out=qk[:], in0=qe[:], in1=ksb[:], op=OP.mult)
            ssum = po.small.tile([P, NH], F32, tag="ssum")
            nc.vector.tensor_reduce(out=ssum[:], in_=qk[:].rearrange("p (h d) -> p h d", h=NH),
                                    axis=AX.X, op=OP.add)
            sev = po.small.tile([P, NH], F32, tag="sev")
            nc.vector.tensor_tensor(out=sev[:], in0=ssum[:],
                                    in1=mb_t[:, col, 1:2].to_broadcast([P, NH]), op=OP.mult)
            pl = po.pl.tile([P, HID + NH], F32, tag="pl")
            nc.scalar.activation(out=pl[:, HID:HID + NH], in_=sev[:], func=AF.Exp, scale=SCALE)
            nc.vector.tensor_tensor(
                out=pl[:, 0:HID].rearrange("p (h d) -> p h d", h=NH),
                in0=kv[:, HID:2 * HID].rearrange("p (h d) -> p h d", h=NH),
                in1=pl[:, HID:HID + NH].rearrange("p (h o) -> p h o", h=NH).to_broadcast([P, NH, HD]),
                op=OP.mult)

            if boundary:
                nc.tensor.matmul(out=agg_tiles[t - 1][:], lhsT=pprev[:], rhs=pl[:],
                                 start=False, stop=True)
            last_contrib = (t == tcnt - 1 and j == K[t] - 1)
            nc.tensor.matmul(out=agg_tiles[t][:], lhsT=pcur[:], rhs=pl[:],
                             start=(j == 0), stop=last_contrib)

        if t > 0:
            # tile t-1 is complete: its boundary consumer was slot (t, 0)
            if K[t] == 0:
                pass
            finalize(t - 1)
            del qt_tiles[t - 1]
            del iota_tiles[t - 1]
    finalize(tcnt - 1)


def _emit_combine(nc, po, h_dram, shard_base, rows_total, deltas, out_dram):
    """out = LN(h + sum(deltas)) per 128-row tile."""
    tcnt = (rows_total + P - 1) // P
    for t in range(tcnt):
        rows = min(P, rows_total - t * P)
        hh = po.work.tile([P, HID], F32, tag="c_h")
        if rows < P:
            nc.vector.memset(hh[:], 0.0)
        nc.sync.dma_start(out=hh[:rows, :], in_=h_dram[shard_base + t * P: shard_base + t * P + rows, :])
        pre = po.work.tile([P, HID], F32, tag="c_pre")
        dts = []
        for dram in deltas:
            dt_ = po.work.tile([P, HID], F32, tag="c_d")
            nc.sync.dma_start(out=dt_[:], in_=dram[t * P:(t + 1) * P, :])
            dts.append(dt_)
        nc.vector.tensor_tensor(out=pre[:], in0=hh[:], in1=dts[0][:], op=OP.add)
        for dt_ in dts[1:]:
            nc.vector.tensor_tensor(out=pre[:], in0=pre[:], in1=dt_[:], op=OP.add)
        nmu = po.small.tile([P, 1], F32, tag="c_mu")
        nc.vector.tensor_reduce(out=nmu[:], in_=pre[:], axis=AX.X, op=OP.add)
        nc.vector.tensor_scalar(out=nmu[:], in0=nmu[:], scalar1=-1.0 / HID,
                                scalar2=None, op0=OP.mult)
        xc = po.work.tile([P, HID], F32, tag="c_xc")
        nc.vector.tensor_scalar(out=xc[:], in0=pre[:], scalar1=nmu[:], scalar2=None, op0=OP.add)
        sq = po.work.tile([P, HID], F32, tag="c_sq")
        nc.vector.tensor_tensor(out=sq[:], in0=xc[:], in1=xc[:], op=OP.mult)
        var = po.small.tile([P, 1], F32, tag="c_var")
        nc.vector.tensor_reduce(out=var[:], in_=sq[:], axis=AX.X, op=OP.add)
        nc.vector.tensor_scalar(out=var[:], in0=var[:], scalar1=1.0 / HID,
                                scalar2=EPS_LN, op0=OP.mult, op1=OP.add)
        rec = po.small.tile([P, 1], F32, tag="c_rec")
        nc.vector.reciprocal(out=rec[:], in_=var[:])
        rst = po.small.tile([P, 1], F32, tag="c_rst")
        nc.scalar.activation(out=rst[:], in_=rec[:], func=AF.Sqrt)
        ot = po.work.tile([P, HID], F32, tag="c_out")
        nc.vector.tensor_scalar(out=ot[:], in0=xc[:], scalar1=rst[:], scalar2=None, op0=OP.mult)
        nc.sync.dma_start(out=out_dram[t * P:(t + 1) * P, :], in_=ot[:])


# ----------------------------------------------------------------------------
# Top level
# ----------------------------------------------------------------------------

def kernel(**inputs):
    from contextlib import ExitStack

    h_user = np.ascontiguousarray(np.asarray(inputs["h_user"], np.float32))
    h_comp = np.ascontiguousarray(np.asarray(inputs["h_comp"], np.float32))
    NU, _ = h_user.shape
    NC_, _ = h_comp.shape
    DU, DC = NU // NCORES, NC_ // NCORES
    TU, TC = (DU + P - 1) // P, (DC + P - 1) // P

    for r in (1, 2, 3):
        assert not np.any(np.asarray(inputs[f"mb{r}"])), "nonzero mb unsupported"
    assert not np.any(np.asarray(inputs["pb_user"])) and not np.any(np.asarray(inputs["pb_comp"]))
    assert np.all(np.asarray(inputs["g_user"]) == 1) and np.all(np.asarray(inputs["g_comp"]) == 1)
    assert not np.any(np.asarray(inputs["b_user"])) and not np.any(np.asarray(inputs["b_comp"]))

    rw = {r: _sigmoid(np.asarray(inputs[f"imp{r}"], np.float32)) for r in (1, 2, 3)}
    w_c = 1.0  # softmax over a single element
    e2, e3 = np.exp(rw[2] - max(rw[2], rw[3])), np.exp(rw[3] - max(rw[2], rw[3]))
    w_u2, w_u3 = e2 / (e2 + e3), e3 / (e2 + e3)
    s_fac = {1: w_c * rw[1], 2: w_u2 * rw[2], 3: w_u3 * rw[3]}

    # host routing / schedules
    rels = []
    for r, dshard in ((1, DC), (2, DU), (3, DU)):
        src = np.asarray(inputs[f"src{r}"], np.int64)
        dst = np.asarray(inputs[f"dst{r}"], np.int64)
        ew = np.asarray(inputs[f"ew{r}"], np.float64)
        K, slots, per_core = _prep_relation(src, dst, ew, dshard, NCORES)
        blocks = _pack_blocks(per_core)
        rels.append({"r": r, "K": K, "blocks": blocks})

    iota_rep = np.tile(np.arange(P, dtype=np.float32)[None, :], (P, 1))
    identity = np.eye(P, dtype=np.float32)

    # ---- build program ----
    nc = bacc.Bacc(None, target_bir_lowering=False, num_devices=NCORES)
    d_hu = nc.declare_dram_parameter("h_user", [NU, HID], F32, isOutput=False)
    d_hc = nc.declare_dram_parameter("h_comp", [NC_, HID], F32, isOutput=False)
    d_iota = nc.declare_dram_parameter("iota_rep", [P, P], F32, isOutput=False)
    d_iden = nc.declare_dram_parameter("identity", [P, P], F32, isOutput=False)
    wraw = {}
    for r in (1, 2, 3):
        for w in ("qw", "kw", "vw", "mw"):
            wraw[f"{w}{r}"] = nc.declare_dram_parameter(f"{w}{r}", [HID, HID], F32, isOutput=False)
    wraw["pw_user"] = nc.declare_dram_parameter("pw_user", [HID, HID], F32, isOutput=False)
    wraw["pw_comp"] = nc.declare_dram_parameter("pw_comp", [HID, HID], F32, isOutput=False)
    d_srcb, d_metab, d_delta = {}, {}, {}
    for rel in rels:
        r = rel["r"]
        bshape = rel["blocks"][0]["srcb"].shape
        mshape = rel["blocks"][0]["metab"].shape
        d_srcb[r] = nc.declare_dram_parameter(f"srcb{r}", list(bshape), I32, isOutput=False)
        d_metab[r] = nc.declare_dram_parameter(f"metab{r}", list(mshape), F32, isOutput=False)
        tcnt = len(rel["K"])
        d_delta[r] = nc.dram_tensor(f"delta{r}", [tcnt * P, HID], F32)
    d_ou = nc.declare_dram_parameter("out_u", [TU * P, HID], F32, isOutput=True)
    d_oc = nc.declare_dram_parameter("out_c", [TC * P, HID], F32, isOutput=True)

    # NOTE: dst shard base differs per core -> bake core id into ... SPMD needs
    # identical program, so shard base must come from the partition id. Instead
    # we pass each core ITS OWN shard of h for dst/residual reads:
    d_hu_shard = nc.declare_dram_parameter("hu_shard", [DU, HID], F32, isOutput=False)
    d_hc_shard = nc.declare_dram_parameter("hc_shard", [DC, HID], F32, isOutput=False)

    with tile.TileContext(nc) as tc:
        with ExitStack() as ctx:
            po = Pools(nc, tc, ctx)
            iden_t = po.const.tile([P, P], F32, tag="iden")
            nc.sync.dma_start(out=iden_t[:], in_=d_iden[:])
            iota_t = po.const.tile([P, P], F32, tag="iota_rep")
            nc.sync.dma_start(out=iota_t[:], in_=d_iota[:])
            wins = {
                "raw": {k: v[:] for k, v in wraw.items()},
                "transpose": ["qw1", "qw2", "qw3", "kw1", "kw2", "kw3",
                              "vw1", "vw2", "vw3", "pw_user", "pw_comp"],
                "mt": [(1, "pw_comp", s_fac[1]), (2, "pw_user", s_fac[2]),
                       (3, "pw_user", s_fac[3])],
            }
            W = _emit_weight_prep(nc, po, iden_t, wins)
            relcfg = {
                1: {"h_src": d_hu, "h_dst": d_hc_shard, "dst_base": 0, "dst_rows": DC},
                2: {"h_src": d_hc, "h_dst": d_hu_shard, "dst_base": 0, "dst_rows": DU},
                3: {"h_src": d_hu, "h_dst": d_hu_shard, "dst_base": 0, "dst_rows": DU},
            }
            for rel in rels:
                r = rel["r"]
                cfg = dict(relcfg[r])
                cfg.update({
                    "K": rel["K"], "qwT": W[f"T_qw{r}"], "kvwT": W[f"kvwT{r}"],
                    "MT": W[f"MT{r}"], "srcb": d_srcb[r], "metab": d_metab[r],
                    "delta": d_delta[r],
                })
                _emit_pass(nc, po, iden_t, iota_t, cfg)
            _emit_combine(nc, po, d_hc_shard, 0, DC, [d_delta[1]], d_oc)
            _emit_combine(nc, po, d_hu_shard, 0, DU, [d_delta[2], d_delta[3]], d_ou)

    nc.compile()

    in_maps = []
    for c in range(NCORES):
        m = {
            "h_user": h_user, "h_comp": h_comp,
            "hu_shard": h_user[c * DU:(c + 1) * DU],
            "hc_shard": h_comp[c * DC:(c + 1) * DC],
            "iota_rep": iota_rep, "identity": identity,
            "pw_user": np.asarray(inputs["pw_user"], np.float32),
            "pw_comp": np.asarray(inputs["pw_comp"], np.float32),
        }
        for r in (1, 2, 3):
            for w in ("qw", "kw", "vw", "mw"):
                m[f"{w}{r}"] = np.asarray(inputs[f"{w}{r}"], np.float32)
        for rel in rels:
            r = rel["r"]
            m[f"srcb{r}"] = rel["blocks"][c]["srcb"]
            m[f"metab{r}"] = rel["blocks"][c]["metab"]
        in_maps.append(m)

    if os.environ.get("KERNEL_USE_SIM"):
        from concourse.bass_interp import CoreSim, MultiCoreSim
        if NCORES == 1:
            sim = CoreSim(nc)
            for k, v in in_maps[0].items():
                sim.tensor(k)[:] = v
            sim.simulate()
            results = [{"out_u": np.array(sim.tensor("out_u")),
                        "out_c": np.array(sim.tensor("out_c"))}]
        else:
            sim = MultiCoreSim(nc, num_cores=NCORES)
            for c in range(NCORES):
                for k, v in in_maps[c].items():
                    sim.cores[c].tensor(k)[:] = v
            sim.simulate()
            results = [{"out_u": np.array(sim.cores[c].mem_tensor("out_u")),
                        "out_c": np.array(sim.cores[c].mem_tensor("out_c"))}
                       for c in range(NCORES)]
        r_obj = None
    else:
        r_obj = run_bass_kernel_spmd(nc, in_maps, list(range(NCORES)),
                                     trace=bool(os.environ.get("KERNEL_TRACE")))
        results = r_obj.results

    out_u = np.concatenate([results[c]["out_u"][:DU] for c in range(NCORES)], axis=0)
    out_c = np.concatenate([results[c]["out_c"][:DC] for c in range(NCORES)], axis=0)
    kernel.last_result = r_obj
    return out_u, out_c
